# revision 1
# baseline (speedup 1.0000x reference)
"""Trainium2 Bass kernel for nn_DeLaNFriction (DeLaN with friction).

Per sample (batch 65536, 7-DOF): three MLPs 7->128->128->128->{28,1,28}
(softplus x3), PSD mass/friction matrices from Cholesky-vector outputs
(exp on diag), analytic grad/JVP/VJP terms, batched 7x7 LDL^T solve.

Sharding: pure data-parallel over batch across 8 cores (8192 samples/core).

Device layouts:
  - MLP + tril algebra: [feature, batch] (feature on partitions), all
    tensors at base partition 0 (2-input DVE ops require equal SBUF base
    partitions; matmul requires equal lhsT/rhs bases).
  - 7x7 LDL solve: [128 samples (partitions), feature, G groups]: per-entry
    ops are [128, 1, G] contiguous vector ops.
Engine plan: PE = all matmuls incl. 0/1-map gather/scatter + transposes;
ACT = softplus/exp passes (sigma = 1-exp(-softplus) trick, two table sets);
DVE = elementwise products, PSUM evacuation, LDL solve.
"""

import numpy as np

import concourse.bass as bass
import concourse.bacc as bacc
import concourse.mybir as mybir
import concourse.tile as tile
from concourse.masks import make_identity

# Force every exp/ln activation onto the one table set that contains both
# (natural_log_exp_and_others); otherwise bacc's table-load pass assigns
# exp->exp_and_others and ln->natural_log and inserts a ~1.3us
# LoadActFuncSet before nearly every activation. Set order/indices must be
# preserved (act_func_set_id is positional in act_info.json).
import concourse.hw_specs as _hw_specs

_orig_get_tables = _hw_specs.get_activation_tables
_EXPLN = {mybir.ActivationFunctionType.Exp, mybir.ActivationFunctionType.Ln}


def _patched_tables(module_arch):
    tabs = _orig_get_tables(module_arch)
    out = {}
    for name, fns in tabs.items():
        if name != "natural_log_exp_and_others":
            fns = fns - _EXPLN
        out[name] = fns
    return out


_hw_specs.get_activation_tables = _patched_tables
import concourse.bacc as _bacc_mod
_bacc_mod.get_activation_tables = _patched_tables

N = 7
HID = 128
NCH = 28
NPAIR = 84
EPS_M = 1e-3
EPS_B = 1e-3
EPS_SOLVE = 1e-4
EBAR = EPS_M + EPS_SOLVE
BATCH = 65536
NCORES = 8
F32 = mybir.dt.float32
F32R = mybir.dt.float32r

# ---- tril entry order: diag-first, then strict lower row-major ----
ENT = [(i, i) for i in range(N)] + [(i, j) for i in range(N) for j in range(i)]
ROWIDX = [e[0] for e in ENT]
COLIDX = [e[1] for e in ENT]
ENT_IDX = {e: k for k, e in enumerate(ENT)}
TRI_I, TRI_J = np.tril_indices(N)
_ref_idx = {(i, j): k for k, (i, j) in enumerate(zip(TRI_I, TRI_J))}
PERM = np.array([_ref_idx[e] for e in ENT])  # ours[k] = ref[PERM[k]]


def _off(i, j):
    """index into strict-lower row-major storage (21 entries)."""
    return i * (i - 1) // 2 + j


def _me(i, j):
    """index of M entry (i>=j) in ENT (diag-first) order."""
    if i < j:
        i, j = j, i
    return i if i == j else N + _off(i, j)


def build_maps():
    Ri = np.zeros((N, NCH), np.float32)
    Rj = np.zeros((N, NCH), np.float32)
    Si = np.zeros((NCH, N), np.float32)
    Sj = np.zeros((NCH, N), np.float32)
    for k in range(NCH):
        Ri[ROWIDX[k], k] = 1
        Rj[COLIDX[k], k] = 1
        Si[k, ROWIDX[k]] = 1
        Sj[k, COLIDX[k]] = 1
    pa, pb, pt = [], [], []
    for t, (i, j) in enumerate(ENT):
        for k in range(j + 1):
            pa.append(ENT_IDX[(i, k)])
            pb.append(ENT_IDX[(j, k)])
            pt.append(t)
    assert len(pa) == NPAIR
    RA = np.zeros((NCH, NPAIR), np.float32)
    RB = np.zeros((NCH, NPAIR), np.float32)
    SM = np.zeros((NPAIR, NCH), np.float32)
    for p in range(NPAIR):
        RA[pa[p], p] = 1
        RB[pb[p], p] = 1
        SM[p, pt[p]] = 1
    EIn = (-EPS_B * np.eye(N)).astype(np.float32)
    return dict(Ri=Ri, Rj=Rj, Si=Si, SiN=-Si, Sj=Sj, RA=RA, RB=RB, SM=SM,
                EIn=EIn)


def build_program(BPC, C, pb3_val, mm_f32r=False, sim_mode=False):
    """Build the per-core Bass program. BPC = batch per core, C = chunk."""
    assert BPC % C == 0 and C % 512 == 0 and BPC % 128 == 0
    nchunks = BPC // C
    G = BPC // 128           # number of 128-sample groups (solve inner dim)
    BLK = C // 128           # groups per chunk
    HREG = min(1024, C)      # psum working width
    nc = bacc.Bacc(None, target_bir_lowering=False)
    AF = mybir.ActivationFunctionType
    OP = mybir.AluOpType

    def mmdt(ap):
        return ap.bitcast(F32R) if mm_f32r else ap

    maps = build_maps()

    # ---------------- DRAM tensors ----------------
    qT = nc.dram_tensor("qT", [N, BPC], F32, kind="ExternalInput")
    qdT = nc.dram_tensor("qdT", [N, BPC], F32, kind="ExternalInput")
    win = {}

    def din(name, shape):
        win[name] = nc.dram_tensor(name, shape, F32, kind="ExternalInput")

    for net in "mpf":
        K3 = 1 if net == "p" else NCH
        din(f"w0{net}", [N, HID])
        din(f"w1{net}", [HID, HID])
        din(f"w2{net}", [HID, HID])
        din(f"w3{net}", [HID, K3])
        for li in range(3):
            din(f"b{li}{net}", [HID, 1])
    din("b3m", [NCH, 1])
    din("b3f", [NCH, 1])
    din("wm3T", [NCH, HID])
    din("wm2T", [HID, HID])
    din("wm1T", [HID, HID])
    din("wm0Tn", [HID, N])
    din("wp3T", [1, HID])
    din("wp3Tn", [1, HID])
    din("wp2T", [HID, HID])
    din("wp1T", [HID, HID])
    din("wp0T", [HID, N])
    for name, arr in maps.items():
        din(name, list(arr.shape))
    out_d = nc.dram_tensor("out", [BPC, N], F32, kind="ExternalOutput")

    with tile.TileContext(nc) as tc:
        with (
            tc.tile_pool(name="const", bufs=1) as cpool,
            tc.tile_pool(name="hpool", bufs=12) as hpool,    # [128, C] rotating
            tc.tile_pool(name="tri", bufs=1) as trip,        # skinny tiles
            tc.tile_pool(name="slv", bufs=1) as slvp,        # solve-layout tiles
            tc.tile_pool(name="pbig", bufs=2, space="PSUM") as pbig,   # [128,1024]
            tc.tile_pool(name="psml", bufs=3, space="PSUM") as psml,   # [128,512]
            tc.tile_pool(name="pacc", bufs=1, space="PSUM") as pacc,   # [7,512]
        ):
            # ---------- constants to SBUF ----------
            W = {}
            for name, t in win.items():
                W[name] = cpool.tile(list(t.shape), F32, name=name, tag=name)
                nc.sync.dma_start(out=W[name][:], in_=t[:])
            ident = cpool.tile([128, 128], F32, tag="ident")
            make_identity(nc, ident[:])
            ebar_col = cpool.tile([NCH, 1], F32, tag="ebar_col")
            nc.vector.memset(ebar_col[:], 0.0)
            nc.vector.memset(ebar_col[0:N, :], EBAR)
            pb3_ap = cpool.tile([1, 1], F32, tag="pb3_ap")
            nc.vector.memset(pb3_ap[:], float(pb3_val))
            ones_col = cpool.tile([HID, 1], F32, tag="ones_col")
            nc.vector.memset(ones_col[:], 1.0)

            def softplus_act(dst, src, bias):
                # this PWP build ships no softplus spline; exp+ln share one
                # table set (natural_log_exp_and_others) with the sigma exps
                nc.scalar.activation(out=dst, in_=src, func=AF.Exp, bias=bias)
                nc.scalar.activation(out=dst, in_=dst, func=AF.Ln,
                                     bias=ones_col[0:dst.shape[0], :])

            # persistent solve-layout tile: 28 M entries + 7 rhs per sample
            MRv = slvp.tile([128, 35, G], F32, tag="MRv")

            for ci in range(nchunks):
                c0 = ci * C

                def tri_tile(nm, rows=NCH):
                    return trip.tile([rows, C], F32, name=nm, tag=nm)

                qs = trip.tile([N, C], F32, name="qs", tag="qs", bufs=2)
                qds = trip.tile([N, C], F32, name="qds", tag="qds", bufs=2)
                nc.sync.dma_start(out=qs[:], in_=qT[:, c0:c0 + C])
                nc.sync.dma_start(out=qds[:], in_=qdT[:, c0:c0 + C])

                def big_mm_act(dst, lhsT_ap, src, bias):
                    """dst = softplus((lhsT.T @ src) + bias) over chunk."""
                    for hof in range(0, C, HREG):
                        pa = pbig.tile([HID, HREG], F32, tag="pa", name="pa")
                        for s in range(0, HREG, 512):
                            nc.tensor.matmul(
                                pa[:, s:s + 512], mmdt(lhsT_ap),
                                mmdt(src[:, hof + s:hof + s + 512]),
                                start=True, stop=True)
                        nc.scalar.activation(out=dst[:, hof:hof + HREG], in_=pa[:],
                                             func=AF.Exp, bias=bias)   # e = exp(a+b)
                        nc.scalar.activation(out=dst[:, hof:hof + HREG],
                                             in_=dst[:, hof:hof + HREG],
                                             func=AF.Ln, bias=ones_col[:])

                def big_mm_stt(dst, lhsT_ap, src, sig_tile):
                    """dst = (sig - 1) * (lhsT.T @ src)  (sign-flipping chain)."""
                    for hof in range(0, C, HREG):
                        pa = pbig.tile([HID, HREG], F32, tag="pa", name="pa")
                        for s in range(0, HREG, 512):
                            nc.tensor.matmul(
                                pa[:, s:s + 512], mmdt(lhsT_ap),
                                mmdt(src[:, hof + s:hof + s + 512]),
                                start=True, stop=True)
                        nc.vector.scalar_tensor_tensor(
                            out=dst[:, hof:hof + HREG], in0=sig_tile[:, hof:hof + HREG],
                            scalar=1.0, in1=pa[:],
                            op0=OP.subtract, op1=OP.mult)

                # ---------- forward MLPs ----------
                h = {}
                for net in "mpf":
                    prev = qs
                    for li in range(3):
                        cur = hpool.tile([HID, C], F32, tag="hbuf", name="hbuf")
                        big_mm_act(cur, W[f"w{li}{net}"][:], prev, W[f"b{li}{net}"][:])
                        h[f"{net}{li}"] = cur
                        prev = cur

                # final layers at 512 width into SBUF (bias via tensor_scalar)
                cm_s = tri_tile("cm_s")
                cf_s = tri_tile("cf_s")
                spyp = tri_tile("spyp", 1)
                for s in range(0, C, 512):
                    py = psml.tile([128, 512], F32, tag="ps", name="ps")
                    nc.tensor.matmul(py[0:NCH, :], mmdt(W["w3m"][:]),
                                     mmdt(h["m2"][:, s:s + 512]), start=True, stop=True)
                    nc.vector.tensor_scalar(out=cm_s[:, s:s + 512], in0=py[0:NCH, :],
                                            scalar1=W["b3m"][:], scalar2=None, op0=OP.add)
                    pyf = psml.tile([128, 512], F32, tag="ps", name="ps")
                    nc.tensor.matmul(pyf[0:NCH, :], mmdt(W["w3f"][:]),
                                     mmdt(h["f2"][:, s:s + 512]), start=True, stop=True)
                    nc.vector.tensor_scalar(out=cf_s[:, s:s + 512], in0=pyf[0:NCH, :],
                                            scalar1=W["b3f"][:], scalar2=None, op0=OP.add)
                    pyp = psml.tile([128, 512], F32, tag="ps", name="ps")
                    nc.tensor.matmul(pyp[0:1, :], mmdt(W["w3p"][:]),
                                     mmdt(h["p2"][:, s:s + 512]), start=True, stop=True)
                    nc.scalar.activation(out=spyp[:, s:s + 512], in_=pyp[0:1, :],
                                         func=AF.Exp, bias=pb3_ap[:])

                # sigma passes (same exp/ln table set), in place over h
                sig = {}
                for key in ("m0", "m1", "m2", "p0", "p1", "p2"):
                    nc.scalar.activation(out=h[key][:], in_=h[key][:], func=AF.Exp,
                                         scale=-1.0)
                    sig[key] = h[key]   # now holds exp(-h) = 1 - sigma
                # sigma(y_p) = e/(1+e) from spyp which holds e_yp
                syp = tri_tile("syp", 1)
                nc.vector.tensor_scalar(out=syp[:], in0=spyp[:], scalar1=1.0,
                                        scalar2=None, op0=OP.add)
                nc.vector.reciprocal_approx_fast(out=syp[:], in_=syp[:])
                nc.vector.tensor_mul(syp[:], syp[:], spyp[:])
                L_ = trip.tile([NCH, C], F32, name="L_", tag="L_", bufs=2)
                Lf_ = tri_tile("Lf_")
                Lones = tri_tile("Lones")
                nc.vector.tensor_copy(L_[:], cm_s[:])
                nc.vector.tensor_copy(Lf_[:], cf_s[:])
                nc.scalar.activation(out=L_[0:N, :], in_=cm_s[0:N, :], func=AF.Exp)
                nc.scalar.activation(out=Lf_[0:N, :], in_=cf_s[0:N, :], func=AF.Exp)
                nc.vector.memset(Lones[:], 1.0)
                nc.vector.tensor_copy(Lones[0:N, :], L_[0:N, :])

                # ---------- jvp through m-net (tangent qd; sign-tracked) ----------
                t0 = hpool.tile([HID, C], F32, tag="hbuf", name="hbuf")
                big_mm_stt(t0, W["w0m"][:], qds, sig["m0"])        # -t_h0
                t1 = hpool.tile([HID, C], F32, tag="hbuf", name="hbuf")
                big_mm_stt(t1, W["w1m"][:], t0, sig["m1"])         # +t_h1
                t2 = hpool.tile([HID, C], F32, tag="hbuf", name="hbuf")
                big_mm_stt(t2, W["w2m"][:], t1, sig["m2"])         # -t_h2
                tL = tri_tile("tL")                                 # -tL true
                for s in range(0, C, 512):
                    ptc = psml.tile([128, 512], F32, tag="ps", name="ps")
                    nc.tensor.matmul(ptc[0:NCH, :], mmdt(W["w3m"][:]),
                                     mmdt(t2[:, s:s + 512]), start=True, stop=True)
                    # tL = t_c * Lones (diag scaling)
                    nc.vector.tensor_mul(tL[:, s:s + 512], ptc[0:NCH, :],
                                         Lones[:, s:s + 512])

                # ---------- tril algebra ----------
                def map_mm(dst, lhs_ap, rhs_ap, Mrows, eng=None):
                    for s in range(0, C, 512):
                        pm = psml.tile([128, 512], F32, tag="ps", name="ps")
                        nc.tensor.matmul(pm[0:Mrows, :], mmdt(lhs_ap),
                                         mmdt(rhs_ap[:, s:s + 512]),
                                         start=True, stop=True)
                        if eng == "act":
                            nc.scalar.copy(dst[:, s:s + 512], pm[0:Mrows, :])
                        else:
                            nc.vector.tensor_copy(dst[:, s:s + 512], pm[0:Mrows, :])

                qd_rep = trip.tile([NCH, C], F32, name="qd_rep", tag="qd_rep", bufs=2)
                map_mm(qd_rep, W["Ri"][:], qds, NCH, eng="act")
                P_u = tri_tile("P_u")
                nc.gpsimd.tensor_mul(P_u[:], L_[:], qd_rep[:])
                u_ = tri_tile("u_", N)
                map_mm(u_, W["Sj"][:], P_u, N)
                u_rep = tri_tile("u_rep")
                map_mm(u_rep, W["Rj"][:], u_, NCH, eng="act")
                g_c = tri_tile("g_c")
                nc.vector.tensor_mul(g_c[:], u_rep[:], qd_rep[:])
                nc.vector.tensor_mul(g_c[:], g_c[:], Lones[:])
                P_v = tri_tile("P_v")                               # -true
                nc.gpsimd.tensor_mul(P_v[:], tL[:], qd_rep[:])
                v_ = tri_tile("v_", N)
                map_mm(v_, W["Sj"][:], P_v, N)                      # -true
                v_rep = tri_tile("v_rep")
                map_mm(v_rep, W["Rj"][:], v_, NCH, eng="act")                  # -true
                dLu = tri_tile("dLu")                               # -true
                nc.gpsimd.tensor_mul(dLu[:], tL[:], u_rep[:])
                Lv = tri_tile("Lv")                                 # -true
                nc.gpsimd.tensor_mul(Lv[:], L_[:], v_rep[:])
                Pf = tri_tile("Pf")
                nc.gpsimd.tensor_mul(Pf[:], Lf_[:], qd_rep[:])
                uf_ = tri_tile("uf_", N)
                map_mm(uf_, W["Sj"][:], Pf, N)
                uf_rep = tri_tile("uf_rep")
                map_mm(uf_rep, W["Rj"][:], uf_, NCH, eng="act")
                Pfr = tri_tile("Pfr")
                nc.gpsimd.tensor_mul(Pfr[:], Lf_[:], uf_rep[:])
                # M entries via 84-pair trick
                Apr = tri_tile("Apr", NPAIR)
                map_mm(Apr, W["RA"][:], L_, NPAIR)
                Mprod = tri_tile("Mprod", NPAIR)
                for s in range(0, C, 512):
                    pb_ = psml.tile([128, 512], F32, tag="ps", name="ps")
                    nc.tensor.matmul(pb_[0:NPAIR, :], mmdt(W["RB"][:]),
                                     mmdt(L_[:, s:s + 512]), start=True, stop=True)
                    nc.vector.tensor_mul(Mprod[:, s:s + 512], Apr[:, s:s + 512],
                                         pb_[0:NPAIR, :])
                M_s = tri_tile("M_s")
                for s in range(0, C, 512):
                    pm = psml.tile([128, 512], F32, tag="ps", name="ps")
                    nc.tensor.matmul(pm[0:NCH, :], mmdt(W["SM"][:]),
                                     mmdt(Mprod[:, s:s + 512]), start=True, stop=True)
                    nc.vector.tensor_scalar(out=M_s[:, s:s + 512], in0=pm[0:NCH, :],
                                            scalar1=ebar_col[:], scalar2=None, op0=OP.add)

                # ---------- vjp m-net (cotangent g_c) ----------
                gm2 = hpool.tile([HID, C], F32, tag="hbuf", name="hbuf")
                for s in range(0, C, 512):   # K=28 first step
                    pg = psml.tile([128, 512], F32, tag="ps", name="ps")
                    nc.tensor.matmul(pg[:], mmdt(W["wm3T"][:]), mmdt(g_c[:, s:s + 512]),
                                     start=True, stop=True)
                    nc.vector.scalar_tensor_tensor(
                        out=gm2[:, s:s + 512], in0=sig["m2"][:, s:s + 512], scalar=1.0,
                        in1=pg[:], op0=OP.subtract, op1=OP.mult)   # -g_a2
                gm1 = hpool.tile([HID, C], F32, tag="hbuf", name="hbuf")
                big_mm_stt(gm1, W["wm2T"][:], gm2, sig["m1"])      # +g_a1
                gm0 = hpool.tile([HID, C], F32, tag="hbuf", name="hbuf")
                big_mm_stt(gm0, W["wm1T"][:], gm1, sig["m0"])      # -g_a0

                # ---------- vjp p-net ----------
                gp2 = hpool.tile([HID, C], F32, tag="hbuf", name="hbuf")
                for s in range(0, C, 512):
                    pg = psml.tile([128, 512], F32, tag="ps", name="ps")
                    sl = slice(s, s + 512)
                    nc.tensor.matmul(pg[:], mmdt(W["wp3T"][:]), mmdt(syp[:, sl]),
                                     start=True, stop=True)        # = +g_h2p
                    nc.vector.scalar_tensor_tensor(
                        out=gp2[:, s:s + 512], in0=sig["p2"][:, s:s + 512], scalar=1.0,
                        in1=pg[:], op0=OP.subtract, op1=OP.mult)   # -g_a2p
                gp1 = hpool.tile([HID, C], F32, tag="hbuf", name="hbuf")
                big_mm_stt(gp1, W["wp2T"][:], gp2, sig["p1"])      # +g_a1p
                gp0 = hpool.tile([HID, C], F32, tag="hbuf", name="hbuf")
                big_mm_stt(gp0, W["wp1T"][:], gp1, sig["p0"])      # -g_a0p

                # ---------- rhs accumulation in PSUM ----------
                rhs_s = tri_tile("rhs_s", N)
                for s in range(0, C, 512):
                    sl = slice(s, s + 512)
                    pr = pacc.tile([N, 512], F32, tag="pr", name="pr")
                    nc.tensor.matmul(pr[:], mmdt(W["EIn"][:]), mmdt(qds[:, sl]),
                                     start=True, stop=False)                 # -eps_b qd
                    nc.tensor.matmul(pr[:], mmdt(W["Si"][:]), mmdt(dLu[:, sl]),
                                     start=False, stop=False)                # -(dL u)
                    nc.tensor.matmul(pr[:], mmdt(W["Si"][:]), mmdt(Lv[:, sl]),
                                     start=False, stop=False)                # -(L v)
                    nc.tensor.matmul(pr[:], mmdt(W["SiN"][:]), mmdt(Pfr[:, sl]),
                                     start=False, stop=False)                # -friction
                    nc.tensor.matmul(pr[:], mmdt(W["wm0Tn"][:]), mmdt(gm0[:, sl]),
                                     start=False, stop=False)                # +gqM
                    nc.tensor.matmul(pr[:], mmdt(W["wp0T"][:]), mmdt(gp0[:, sl]),
                                     start=False, stop=True)                 # -gqV
                    nc.vector.tensor_copy(rhs_s[:, sl], pr[:])

                # ---------- transpose M,rhs into solve layout ----------
                for b in range(BLK):
                    blk = ci * BLK + b
                    ptr = psml.tile([128, 512], F32, tag="ps", name="ps")
                    nc.tensor.transpose(ptr[:, 0:NCH], M_s[:, b * 128:(b + 1) * 128],
                                        ident[0:NCH, 0:NCH])
                    nc.tensor.transpose(ptr[:, NCH:35], rhs_s[:, b * 128:(b + 1) * 128],
                                        ident[0:N, 0:N])
                    nc.vector.tensor_copy(MRv[:, :, blk], ptr[:, 0:35])

            # ================= LDL^T solve (windowed) =================
            SUB = mybir.AluOpType.subtract

            def solve_window(g0, g1):
                GW = g1 - g0
                MR = MRv[:, :, g0:g1]
                Ltv = slvp.tile([128, 21, GW], F32, name="Ltv", tag="Ltv", bufs=1)
                Cv = slvp.tile([128, 21, GW], F32, name="Cv", tag="Cv", bufs=1)
                Dv = slvp.tile([128, N, GW], F32, name="Dv", tag="Dv", bufs=1)
                rDv = slvp.tile([128, N, GW], F32, name="rDv", tag="rDv", bufs=1)
                scr = slvp.tile([128, 1, GW], F32, name="scr", tag="scr", bufs=2)
                scr2 = slvp.tile([128, 1, GW], F32, name="scr2", tag="scr2", bufs=2)

                def Ms(i, j):
                    k = _me(i, j)
                    return MR[:, k:k + 1, :]

                def lt(i, j):
                    k = _off(i, j)
                    return Ltv[:, k:k + 1, :]

                def cvs(i, j):
                    k = _off(i, j)
                    return Cv[:, k:k + 1, :]

                for j in range(N):
                    dj = Dv[:, j:j + 1, :]
                    if j == 0:
                        nc.vector.tensor_copy(dj, Ms(0, 0))
                    else:
                        nc.vector.tensor_mul(scr[:], lt(j, 0), cvs(j, 0))
                        for k in range(1, j):
                            nc.vector.tensor_mul(scr2[:], lt(j, k), cvs(j, k))
                            nc.vector.tensor_add(scr[:], scr[:], scr2[:])
                        nc.vector.tensor_tensor(out=dj, in0=Ms(j, j), in1=scr[:], op=SUB)
                    nc.vector.reciprocal_approx_accurate(out=rDv[:, j:j + 1, :], in_=dj,
                                                         scratch=scr2[:])
                    for i in range(j + 1, N):
                        cij = cvs(i, j)
                        if j == 0:
                            nc.vector.tensor_copy(cij, Ms(i, 0))
                        else:
                            nc.vector.tensor_mul(scr[:], lt(i, 0), cvs(j, 0))
                            for k in range(1, j):
                                nc.vector.tensor_mul(scr2[:], lt(i, k), cvs(j, k))
                                nc.vector.tensor_add(scr[:], scr[:], scr2[:])
                            nc.vector.tensor_tensor(out=cij, in0=Ms(i, j), in1=scr[:],
                                                    op=SUB)
                        nc.vector.tensor_mul(lt(i, j), cij, rDv[:, j:j + 1, :])

                zv = slvp.tile([128, N, GW], F32, name="zv", tag="zv", bufs=1)
                for i in range(N):
                    zi = zv[:, i:i + 1, :]
                    rhs_i = MR[:, NCH + i:NCH + i + 1, :]
                    if i == 0:
                        nc.vector.tensor_copy(zi, rhs_i)
                    else:
                        nc.vector.tensor_mul(scr[:], lt(i, 0), zv[:, 0:1, :])
                        for k in range(1, i):
                            nc.vector.tensor_mul(scr2[:], lt(i, k), zv[:, k:k + 1, :])
                            nc.vector.tensor_add(scr[:], scr[:], scr2[:])
                        nc.vector.tensor_tensor(out=zi, in0=rhs_i, in1=scr[:], op=SUB)
                wv = slvp.tile([128, N, GW], F32, name="wv", tag="wv", bufs=1)
                nc.vector.tensor_mul(wv[:], zv[:], rDv[:])
                xv = slvp.tile([128, N, GW], F32, name="xv", tag="xv", bufs=2)
                for i in reversed(range(N)):
                    xi = xv[:, i:i + 1, :]
                    if i == N - 1:
                        nc.vector.tensor_copy(xi, wv[:, i:i + 1, :])
                    else:
                        nc.vector.tensor_mul(scr[:], lt(i + 1, i), xv[:, i + 1:i + 2, :])
                        for k in range(i + 2, N):
                            nc.vector.tensor_mul(scr2[:], lt(k, i), xv[:, k:k + 1, :])
                            nc.vector.tensor_add(scr[:], scr[:], scr2[:])
                        nc.vector.tensor_tensor(out=xi, in0=wv[:, i:i + 1, :],
                                                in1=scr[:], op=SUB)
                out_ap = out_d[:].rearrange("(g p) i -> p i g", p=128)[:, :, g0:g1]
                nc.sync.dma_start(out=out_ap, in_=xv[:])

            for w0 in range(0, G, 4 * BLK):
                solve_window(w0, min(w0 + 4 * BLK, G))

    nc.compile()
    return nc


def host_prep(inputs, BPC):
    """Split/transform full inputs into per-core in_maps."""
    q = np.ascontiguousarray(inputs["q"], dtype=np.float32)
    qd = np.ascontiguousarray(inputs["qdot"], dtype=np.float32)
    B = q.shape[0]
    ncores = B // BPC
    common = {}
    for net in "mpf":
        common[f"w0{net}"] = np.ascontiguousarray(inputs[f"{net}W0"], dtype=np.float32)
        common[f"w1{net}"] = np.ascontiguousarray(inputs[f"{net}W1"], dtype=np.float32)
        common[f"w2{net}"] = np.ascontiguousarray(inputs[f"{net}W2"], dtype=np.float32)
        for li in range(3):
            common[f"b{li}{net}"] = np.ascontiguousarray(
                np.asarray(inputs[f"{net}b{li}"], dtype=np.float32).reshape(HID, 1))
    mW3p = np.asarray(inputs["mW3"], dtype=np.float32)[:, PERM]
    fW3p = np.asarray(inputs["fW3"], dtype=np.float32)[:, PERM]
    common["w3m"] = np.ascontiguousarray(mW3p)
    common["w3f"] = np.ascontiguousarray(fW3p)
    common["w3p"] = np.ascontiguousarray(inputs["pW3"], dtype=np.float32)
    common["b3m"] = np.ascontiguousarray(
        np.asarray(inputs["mb3"], dtype=np.float32)[PERM].reshape(NCH, 1))
    common["b3f"] = np.ascontiguousarray(
        np.asarray(inputs["fb3"], dtype=np.float32)[PERM].reshape(NCH, 1))
    common["wm3T"] = np.ascontiguousarray(mW3p.T)
    common["wm2T"] = np.ascontiguousarray(np.asarray(inputs["mW2"], np.float32).T)
    common["wm1T"] = np.ascontiguousarray(np.asarray(inputs["mW1"], np.float32).T)
    common["wm0Tn"] = np.ascontiguousarray(-np.asarray(inputs["mW0"], np.float32).T)
    common["wp3T"] = np.ascontiguousarray(np.asarray(inputs["pW3"], np.float32).T)
    common["wp3Tn"] = np.ascontiguousarray(-np.asarray(inputs["pW3"], np.float32).T)
    common["wp2T"] = np.ascontiguousarray(np.asarray(inputs["pW2"], np.float32).T)
    common["wp1T"] = np.ascontiguousarray(np.asarray(inputs["pW1"], np.float32).T)
    common["wp0T"] = np.ascontiguousarray(np.asarray(inputs["pW0"], np.float32).T)
    for k, v in build_maps().items():
        common[k] = np.ascontiguousarray(v)
    in_maps = []
    for c in range(ncores):
        m = dict(common)
        m["qT"] = np.ascontiguousarray(q[c * BPC:(c + 1) * BPC].T)
        m["qdT"] = np.ascontiguousarray(qd[c * BPC:(c + 1) * BPC].T)
        in_maps.append(m)
    return in_maps


def kernel(**inputs):
    BPC = BATCH // NCORES
    pb3 = float(np.asarray(inputs["pb3"]).reshape(-1)[0])
    nc = build_program(BPC, 1024, pb3, mm_f32r=False)
    in_maps = host_prep(inputs, BPC)
    from concourse.bass_utils import run_bass_kernel_spmd
    res = run_bass_kernel_spmd(nc, in_maps, core_ids=list(range(NCORES)))
    outs = [np.asarray(res.results[c]["out"]) for c in range(NCORES)]
    return np.concatenate(outs, axis=0).astype(np.float32)

